# revision 2
# baseline (speedup 1.0000x reference)
"""GAT layer (PyG GATConv-style) on 8 Trainium2 NeuronCores.

Perf notes (TimelineSim 488567 ns, was 725674 ns):
- Batched log-softmax finalize (FIN=25 blocks per Exp/Ln group) eliminates
  per-block activation-table thrash (Exp<->Ln reloads cost 1283 ns each).
- One DMA per edge block for both 128-row f-halves of the contiguous
  [256, L*P] x-stream region (3-dim AP, >=512B elements keeps full DMA rate).
- Blocked a_dst feature layout [j][f][k][n] gives 512B contiguous runs
  (one DMA per block at full rate instead of two at the sub-512B rate).
- Tuned pools: BX=3, NG=1, BH=5, p2f bufs=4.

Strategy:
- Nodes sharded across 8 cores by destination; edges partitioned by destination
  node (per the sharding hint) in a partition-aligned layout: each destination
  node owns one SBUF partition of its block; its incoming edges sit along the
  free dim, padded to a per-block uniform length L (degree-sorted bin packing
  keeps padding ~1-2%).
- Since h[src] = x[src] @ W.T is linear, the source-feature exchange is done by
  expanding x[src] per edge slot on the host (sharding-time data movement);
  the device computes per-edge [h | a_src] with one matmul against the folded
  weight matrix Wt_ext = [W.T | W.T @ att_src-blockdiag]. No device-side
  gather or collective is needed.
- Segment softmax over incoming edges is then core-local: a_dst is a
  per-partition broadcast, exp(leaky(z)) = max(exp(z), exp(0.2 z)) on ACT,
  and the weighted aggregation is a PSUM-accumulated identity matmul.

kernel(**inputs) takes FULL inputs and returns the FULL [N, 64] output.
"""

import numpy as np
import ml_dtypes

import concourse.bass as bass
import concourse.bacc as bacc
import concourse.tile as tile
from concourse import mybir
from concourse.bass_utils import run_bass_kernel_spmd
from concourse.masks import make_identity

# Problem shape (hardcoded per contract)
N, F, E = 100000, 256, 1600000
H, C = 8, 8
HC = H * C  # 64
NEG_SLOPE = 0.2

P = 128
NCORES = 8
NB = 98                      # blocks per core
NPC = NB * P                 # 12544 node slots per core
NSLOT = NCORES * NPC         # 100352 >= N
TD = HC + H                  # 72: [h(64) | a_src(8)]
PAD_ASRC = -200.0
FIN = 25                     # finalize batch (dst blocks)

bf16 = ml_dtypes.bfloat16


def _host_prep(x, edge_index, W, att_src, att_dst, bias):
    src = np.asarray(edge_index[0], dtype=np.int64)
    dst = np.asarray(edge_index[1], dtype=np.int64)
    loop = np.arange(N, dtype=np.int64)
    src = np.concatenate([src, loop])
    dst = np.concatenate([dst, loop])

    deg = np.bincount(dst, minlength=N).astype(np.int64)

    # nodes sorted by degree desc -> global 128-slot blocks dealt round-robin
    # to cores so every core's j-th block has (nearly) equal max degree.
    order = np.argsort(-deg, kind="stable")
    ks = np.arange(NSLOT)
    g = ks // P
    p = ks % P
    c = g % NCORES
    j = g // NCORES
    rows = c * NPC + j * P + p          # device row of global sorted slot k
    row2node = np.full(NSLOT, -1, dtype=np.int64)
    row2node[rows[:N]] = order
    node2row = np.empty(N, dtype=np.int64)
    node2row[order] = rows[:N]

    # per-core-block uniform L schedule (exact max over the 8-block group)
    deg_slot = np.zeros(NSLOT, dtype=np.int64)
    deg_slot[:N] = deg[order]           # degree of global sorted slot k
    degb = deg_slot.reshape(NSLOT // P, P).max(axis=1)   # per global block g
    L_sched = degb.reshape(NB, NCORES).max(axis=1)
    L_sched = np.maximum(L_sched, 1)
    off = np.zeros(NB + 1, dtype=np.int64)
    off[1:] = np.cumsum(P * L_sched)
    S = int(off[-1])                    # edge slots per core

    # folded weights
    Wt = np.asarray(W, dtype=np.float64).T            # [256, 64]
    att_s = np.asarray(att_src, np.float64)           # [8, 8]
    att_d = np.asarray(att_dst, np.float64)
    Wts = np.stack([Wt[:, h * C:(h + 1) * C] @ att_s[h] for h in range(H)], axis=1)  # [256, 8]
    Wtd = np.stack([Wt[:, h * C:(h + 1) * C] @ att_d[h] for h in range(H)], axis=1)  # [256, 8]
    Wt_ext = np.concatenate([Wt, Wts], axis=1)        # [256, 72]

    # padding-slot input vector: v @ Wts = PAD_ASRC for every head. (h(v) is
    # then nonzero but bounded; exp(PAD_ASRC * slope) kills its contribution.)
    tgt = np.full(H, PAD_ASRC)
    v_pad, *_ = np.linalg.lstsq(Wts.T, tgt, rcond=None)      # [256]
    assert np.abs(Wts.T @ v_pad - tgt).max() < 1e-6

    # edge -> slot (vectorized); slot storage order (j, l, p)
    eorder = np.argsort(dst, kind="stable")
    dst_s = dst[eorder]
    src_s = src[eorder]
    starts = np.zeros(N + 1, dtype=np.int64)
    starts[1:] = np.cumsum(deg)
    l_rank = np.arange(len(dst_s), dtype=np.int64) - starts[dst_s]
    r = node2row[dst_s]
    ec = r // NPC
    within = r % NPC
    ej = within // P
    ep = within % P
    pos = off[ej] + l_rank * P + ep

    x_bf = np.asarray(x, np.float32).astype(bf16)
    v_bf = v_pad.astype(np.float32).astype(bf16)

    # per-core expanded inputs
    in_maps = []
    Wt_ext_bf = np.ascontiguousarray(Wt_ext.astype(np.float32).astype(bf16))
    Wtd_bf = np.ascontiguousarray(Wtd.astype(np.float32).astype(bf16))
    bias_rep = np.tile(np.asarray(bias, np.float32).reshape(1, HC), (P, 1))
    for cc in range(NCORES):
        m = ec == cc
        xe = np.broadcast_to(v_bf, (S, F)).copy()     # pad slots -> v_pad
        xe[pos[m]] = x_bf[src_s[m]]
        # lhsT layout per block: [L][k][128f][128slots]
        parts = []
        for jj in range(NB):
            Lj = int(L_sched[jj])
            a = xe[off[jj]:off[jj + 1]].reshape(Lj, P, F)      # [l, p, f]
            a = a.transpose(2, 0, 1)                           # [f, l, p] = [k,f128,l,p]
            parts.append(np.ascontiguousarray(a).reshape(-1))
        xeT = np.concatenate(parts)
        del xe

        # own-node x for a_dst, blocked layout [j][f(128)][k(2)][n(128)]
        rr = row2node[cc * NPC:(cc + 1) * NPC]
        mm = rr >= 0
        xo = np.zeros((NPC, F), dtype=bf16)
        xo[mm] = x_bf[rr[mm]]
        xoT = np.ascontiguousarray(
            xo.reshape(NB, P, 2, P).transpose(0, 3, 2, 1))  # [NB, f, k, n]

        in_maps.append({
            "xeT": xeT,
            "xoT": xoT,
            "Wt_ext": Wt_ext_bf,
            "Wtd": Wtd_bf,
            "bias_rep": bias_rep,
        })
    return in_maps, L_sched, S, row2node


def _build_program(L_sched, S, reps=1, BX=3, BH=5, BA=2, B2=3, NG=1):
    nc = bacc.Bacc("TRN2", target_bir_lowering=False, debug=False,
                   enable_asserts=False, num_devices=NCORES)
    dt = mybir.dt

    xeT = nc.dram_tensor("xeT", [S * 2 * P], dt.bfloat16, kind="ExternalInput").ap()
    xoT = nc.dram_tensor("xoT", [NB, P, 2 * P], dt.bfloat16, kind="ExternalInput").ap()
    Wt_ext = nc.dram_tensor("Wt_ext", [F, TD], dt.bfloat16, kind="ExternalInput").ap()
    Wtd = nc.dram_tensor("Wtd", [F, H], dt.bfloat16, kind="ExternalInput").ap()
    bias_rep = nc.dram_tensor("bias_rep", [P, HC], dt.float32, kind="ExternalInput").ap()
    out = nc.dram_tensor("out", [NPC, HC], dt.float32, kind="ExternalOutput").ap()

    AF = mybir.ActivationFunctionType
    OP = mybir.AluOpType
    GP7 = 7  # l-groups of 7 share one 504-col psum bank

    with tile.TileContext(nc) as tc:
        with (
            tc.tile_pool(name="const", bufs=1) as constp,
            tc.tile_pool(name="resid", bufs=1) as residp,
        ):
            wt0 = constp.tile([P, TD], dt.bfloat16)
            nc.sync.dma_start(wt0[:], Wt_ext[0:P, :])
            wt1 = constp.tile([P, TD], dt.bfloat16)
            nc.sync.dma_start(wt1[:], Wt_ext[P:2 * P, :])
            wtd0 = constp.tile([P, H], dt.bfloat16)
            nc.sync.dma_start(wtd0[:], Wtd[0:P, :])
            wtd1 = constp.tile([P, H], dt.bfloat16)
            nc.sync.dma_start(wtd1[:], Wtd[P:2 * P, :])
            bias_t = constp.tile([P, HC], dt.float32)
            nc.sync.dma_start(bias_t[:], bias_rep[:])
            ident = constp.tile([P, P], dt.bfloat16)
            make_identity(nc, ident[:])

            adst_own = residp.tile([P, NB * H], dt.bfloat16)

            for _rep in range(reps):

                # ---------------- phase 1: a_dst for own nodes ----------------
                with (
                    tc.tile_pool(name="p1", bufs=3) as p1,
                    tc.tile_pool(name="p1ps", bufs=2, space="PSUM") as p1ps,
                ):
                    for t in range(NB):
                        xot = p1.tile([P, 2 * P], dt.bfloat16, tag="xot")
                        nc.sync.dma_start(xot[:], xoT[t])
                        aps = p1ps.tile([P, H], dt.float32, space="PSUM")
                        nc.tensor.matmul(aps[:], lhsT=xot[:, 0:P], rhs=wtd0[:], start=True, stop=False)
                        nc.tensor.matmul(aps[:], lhsT=xot[:, P:2 * P], rhs=wtd1[:], start=False, stop=True)
                        with nc.allow_low_precision(reason="bf16 a_dst store; fp32 accum in PSUM"):
                            nc.vector.tensor_copy(out=adst_own[:, t * H:(t + 1) * H], in_=aps[:])

                # ---------------- phase 2: edge blocks ----------------
                with (
                    tc.tile_pool(name="p2x", bufs=BX) as p2x,
                    tc.tile_pool(name="p2", bufs=B2) as p2,
                    tc.tile_pool(name="p2f", bufs=4) as p2f,
                    tc.tile_pool(name="heps", bufs=BH, space="PSUM") as hepsp,
                    tc.tile_pool(name="aggps", bufs=BA, space="PSUM") as aggpsp,
                ):
                    xoff = 0
                    fin0 = 0
                    vst = None
                    CH = NG * GP7  # l-chunk: at most NG heps psum banks live at a time
                    for jb in range(NB):
                        L = int(L_sched[jb])
                        LP = L * P
                        xt = p2x.tile([P, 2 * LP], dt.bfloat16, tag="xt")
                        # one DMA for both 128-row f-halves of the contiguous
                        # [256, L*P] block: dims (p, k, c)
                        nc.sync.dma_start(
                            xt[:].rearrange("p (k c) -> p k c", k=2),
                            xeT[xoff:xoff + 2 * P * LP]
                                .rearrange("(k p c) -> p k c", k=2, p=P))
                        xta = xt[:, 0:LP]
                        xtb = xt[:, LP:2 * LP]
                        xoff += 2 * P * LP
                        agg = aggpsp.tile([P, TD], dt.float32, space="PSUM", tag="agg")
                        for ch0 in range(0, L, CH):
                            cl = min(CH, L - ch0)
                            ngrp = (cl + GP7 - 1) // GP7
                            heps = []
                            for grp in range(ngrp):
                                gl = min(GP7, cl - grp * GP7)
                                ps = hepsp.tile([P, gl * TD], dt.float32, space="PSUM", tag="heps")
                                heps.append((ps, gl))
                            logits = p2.tile([P, cl, H], dt.float32, tag="logits")
                            for grp, (ps, gl) in enumerate(heps):
                                for li in range(gl):
                                    l = ch0 + grp * GP7 + li
                                    nc.tensor.matmul(ps[:, li * TD:(li + 1) * TD],
                                                     lhsT=xta[:, l * P:(l + 1) * P], rhs=wt0[:],
                                                     start=(li == 0), stop=False, skip_group_check=True)
                                    nc.tensor.matmul(ps[:, li * TD:(li + 1) * TD],
                                                     lhsT=xtb[:, l * P:(l + 1) * P], rhs=wt1[:],
                                                     start=False, stop=(li == gl - 1), skip_group_check=True)
                                # logits[l, h] = asrc + adst (adst bcast over l)
                                nc.vector.tensor_tensor(
                                    out=logits[:, grp * GP7:grp * GP7 + gl, :],
                                    in0=ps[:].rearrange("p (l d) -> p l d", d=TD)[:, :, HC:TD],
                                    in1=adst_own[:, jb * H:(jb + 1) * H].unsqueeze(1).to_broadcast([P, gl, H]),
                                    op=OP.add)
                            # exp(leaky(z)) = max(exp(z), exp(0.2 z))
                            ex1 = p2.tile([P, cl, H], dt.bfloat16, tag="ex1")
                            nc.scalar.activation(ex1[:], logits[:], AF.Exp)
                            ex2 = p2.tile([P, cl, H], dt.bfloat16, tag="ex2")
                            nc.scalar.activation(ex2[:], logits[:], AF.Exp, scale=NEG_SLOPE)
                            w = p2.tile([P, cl, TD], dt.bfloat16, tag="w")
                            nc.vector.tensor_tensor(out=w[:, :, HC:TD], in0=ex1[:], in1=ex2[:], op=OP.max)
                            # w[:, :, 0:64] = h * expe (per-head broadcast)
                            for grp, (ps, gl) in enumerate(heps):
                                nc.vector.tensor_tensor(
                                    out=w[:, grp * GP7:grp * GP7 + gl, 0:HC].rearrange("p l (h c) -> p l h c", c=C),
                                    in0=ps[:].rearrange("p (l d) -> p l d", d=TD)[:, :, 0:HC].rearrange("p l (h c) -> p l h c", c=C),
                                    in1=w[:, grp * GP7:grp * GP7 + gl, HC:TD].unsqueeze(3).to_broadcast([P, gl, H, C]),
                                    op=OP.mult)
                            # aggregate: psum += I.T @ w_l  -> [m | s]
                            for li in range(cl):
                                l = ch0 + li
                                nc.tensor.matmul(agg[:], lhsT=ident[:], rhs=w[:, li, :],
                                                 start=(l == 0), stop=(l == L - 1), skip_group_check=True)
                        # finalize: v = m/s + bias staged; log_softmax batched
                        if jb % FIN == 0:
                            fin0 = jb
                            kf = min(FIN, NB - fin0)
                            vst = p2f.tile([P, kf * HC], dt.float32, tag="vst")
                        jl = jb - fin0
                        srecip = p2f.tile([P, H], dt.float32, tag="srecip")
                        nc.vector.reciprocal(srecip[:], agg[:, HC:TD])
                        onorm = p2f.tile([P, HC], dt.float32, tag="onorm")
                        nc.vector.tensor_tensor(
                            out=onorm[:].rearrange("p (h c) -> p h c", c=C),
                            in0=agg[:, 0:HC].rearrange("p (h c) -> p h c", c=C),
                            in1=srecip[:].unsqueeze(2).to_broadcast([P, H, C]),
                            op=OP.mult)
                        nc.gpsimd.tensor_tensor(
                            out=vst[:, jl * HC:(jl + 1) * HC],
                            in0=onorm[:], in1=bias_t[:], op=OP.add)
                        if jb == NB - 1 or (jb + 1) % FIN == 0:
                            kf = jb - fin0 + 1
                            et = p2f.tile([P, kf * HC], dt.bfloat16, tag="et")
                            nc.scalar.activation(et[:], vst[:, 0:kf * HC], AF.Exp)
                            sm = p2f.tile([P, kf], dt.float32, tag="sm")
                            nc.vector.tensor_reduce(
                                sm[:].unsqueeze(2),
                                et[:].rearrange("p (f d) -> p f d", d=HC),
                                axis=mybir.AxisListType.X, op=OP.add)
                            ln = p2f.tile([P, kf], dt.float32, tag="ln")
                            nc.scalar.activation(ln[:], sm[:], AF.Ln)
                            fin = p2f.tile([P, kf * HC], dt.float32, tag="fin")
                            nc.vector.tensor_tensor(
                                out=fin[:].rearrange("p (f d) -> p f d", d=HC),
                                in0=vst[:, 0:kf * HC].rearrange("p (f d) -> p f d", d=HC),
                                in1=ln[:].unsqueeze(2).to_broadcast([P, kf, HC]),
                                op=OP.subtract)
                            nc.sync.dma_start(
                                out[fin0 * P:(jb + 1) * P, :]
                                    .rearrange("(f p) d -> p f d", p=P),
                                fin[:].rearrange("p (f d) -> p f d", d=HC))

    nc.compile()
    return nc


def kernel(x, edge_index, W, att_src, att_dst, bias):
    in_maps, L_sched, S, row2node = _host_prep(x, edge_index, W, att_src, att_dst, bias)
    nc = _build_program(L_sched, S)
    res = run_bass_kernel_spmd(nc, in_maps, core_ids=list(range(NCORES)))
    out_full = np.empty((N, HC), dtype=np.float32)
    for cc in range(NCORES):
        o = res.results[cc]["out"]
        rr = row2node[cc * NPC:(cc + 1) * NPC]
        m = rr >= 0
        out_full[rr[m]] = o[m]
    return out_full



# revision 10
# speedup vs baseline: 1.4672x; 1.4672x over previous
"""GAT layer (PyG GATConv-style) on 8 Trainium2 NeuronCores.

Perf notes (TimelineSim 332992 ns, was 725674 ns):
- Batched log-softmax finalize (FIN=25 blocks per Exp/Ln group) eliminates
  per-block activation-table thrash (Exp<->Ln reloads cost 1283 ns each).
- One DMA per edge block for both 128-row f-halves of the contiguous
  [256, L*P] x-stream region (3-dim AP, >=512B elements keeps full DMA rate).
- Blocked a_dst feature layout [j][f][k][n] gives 512B contiguous runs
  (one DMA per block at full rate instead of two at the sub-512B rate).
- Tuned pools: BX=3, NG=1, BH=5, B2=4, p2f bufs=4.
- Phase 1 (a_dst) batched 8 blocks per DMA/psum-group/copy (was 98 tiny
  DMA+copy rounds costing ~86us of wall for 18us of data).

Strategy:
- Nodes sharded across 8 cores by destination; edges partitioned by destination
  node (per the sharding hint) in a partition-aligned layout: each destination
  node owns one SBUF partition of its block; its incoming edges sit along the
  free dim, padded to a per-block uniform length L (degree-sorted bin packing
  keeps padding ~1-2%).
- Since h[src] = x[src] @ W.T is linear, the source-feature exchange is done by
  expanding x[src] per edge slot on the host (sharding-time data movement);
  the device computes per-edge [h | a_src] with one matmul against the folded
  weight matrix Wt_ext = [W.T | W.T @ att_src-blockdiag]. No device-side
  gather or collective is needed.
- Segment softmax over incoming edges is then core-local: a_dst is a
  per-partition broadcast, exp(leaky(z)) = max(exp(z), exp(0.2 z)) on ACT,
  and the weighted aggregation is a PSUM-accumulated identity matmul.

kernel(**inputs) takes FULL inputs and returns the FULL [N, 64] output.
"""

import numpy as np
import ml_dtypes

import concourse.bass as bass
import concourse.bacc as bacc
import concourse.tile as tile
from concourse import mybir
from concourse.bass_utils import run_bass_kernel_spmd
from concourse.masks import make_identity

# Problem shape (hardcoded per contract)
N, F, E = 100000, 256, 1600000
H, C = 8, 8
HC = H * C  # 64
NEG_SLOPE = 0.2

P = 128
NCORES = 8
NB = 98                      # blocks per core
NPC = NB * P                 # 12544 node slots per core
NSLOT = NCORES * NPC         # 100352 >= N
TD = HC + H                  # 72: [h(64) | a_src(8)]
PAD_ASRC = -40.0
FIN = 25                     # finalize batch (dst blocks)

bf16 = ml_dtypes.bfloat16
fp8 = ml_dtypes.float8_e4m3


def _host_prep(x, edge_index, W, att_src, att_dst, bias):
    src = np.asarray(edge_index[0], dtype=np.int64)
    dst = np.asarray(edge_index[1], dtype=np.int64)
    loop = np.arange(N, dtype=np.int64)
    src = np.concatenate([src, loop])
    dst = np.concatenate([dst, loop])

    deg = np.bincount(dst, minlength=N).astype(np.int64)

    # nodes sorted by degree desc -> global 128-slot blocks dealt round-robin
    # to cores so every core's j-th block has (nearly) equal max degree.
    order = np.argsort(-deg, kind="stable")
    ks = np.arange(NSLOT)
    g = ks // P
    p = ks % P
    c = g % NCORES
    j = g // NCORES
    rows = c * NPC + j * P + p          # device row of global sorted slot k
    row2node = np.full(NSLOT, -1, dtype=np.int64)
    row2node[rows[:N]] = order
    node2row = np.empty(N, dtype=np.int64)
    node2row[order] = rows[:N]

    # per-core-block uniform L schedule (exact max over the 8-block group)
    deg_slot = np.zeros(NSLOT, dtype=np.int64)
    deg_slot[:N] = deg[order]           # degree of global sorted slot k
    degb = deg_slot.reshape(NSLOT // P, P).max(axis=1)   # per global block g
    L_sched = degb.reshape(NB, NCORES).max(axis=1)
    L_sched = np.maximum(L_sched, 1)
    off = np.zeros(NB + 1, dtype=np.int64)
    off[1:] = np.cumsum(P * L_sched)
    S = int(off[-1])                    # edge slots per core

    # folded weights
    Wt = np.asarray(W, dtype=np.float64).T            # [256, 64]
    att_s = np.asarray(att_src, np.float64)           # [8, 8]
    att_d = np.asarray(att_dst, np.float64)
    Wts = np.stack([Wt[:, h * C:(h + 1) * C] @ att_s[h] for h in range(H)], axis=1)  # [256, 8]
    Wtd = np.stack([Wt[:, h * C:(h + 1) * C] @ att_d[h] for h in range(H)], axis=1)  # [256, 8]
    Wt_ext = np.concatenate([Wt, Wts], axis=1)        # [256, 72]

    # padding-slot input vector: v @ Wts = PAD_ASRC for every head. (h(v) is
    # then nonzero but bounded; exp(PAD_ASRC * slope) kills its contribution.)
    tgt = np.full(H, PAD_ASRC)
    v_pad, *_ = np.linalg.lstsq(Wts.T, tgt, rcond=None)      # [256]
    assert np.abs(Wts.T @ v_pad - tgt).max() < 1e-6

    # edge -> slot (vectorized); slot storage order (j, l, p)
    eorder = np.argsort(dst, kind="stable")
    dst_s = dst[eorder]
    src_s = src[eorder]
    starts = np.zeros(N + 1, dtype=np.int64)
    starts[1:] = np.cumsum(deg)
    l_rank = np.arange(len(dst_s), dtype=np.int64) - starts[dst_s]
    r = node2row[dst_s]
    ec = r // NPC
    within = r % NPC
    ej = within // P
    ep = within % P
    pos = off[ej] + l_rank * P + ep

    x_bf = np.asarray(x, np.float32).astype(bf16)
    x_f8 = np.asarray(x, np.float32).astype(fp8)
    v_f8 = v_pad.astype(np.float32).astype(fp8)
    assert np.abs(v_pad).max() < 230, np.abs(v_pad).max()

    # per-core expanded inputs
    in_maps = []
    Wt_ext_f8 = np.ascontiguousarray(Wt_ext.astype(np.float32).astype(fp8))
    assert np.abs(Wt_ext).max() < 230
    Wtd_bf = np.ascontiguousarray(Wtd.astype(np.float32).astype(bf16))
    bias_rep = np.tile(np.asarray(bias, np.float32).reshape(1, HC), (P, 1))
    for cc in range(NCORES):
        m = ec == cc
        xe = np.broadcast_to(v_f8, (S, F)).copy()     # pad slots -> v_pad
        xe[pos[m]] = x_f8[src_s[m]]
        # lhsT layout per block: [L][k][128f][128slots]
        parts = []
        for jj in range(NB):
            Lj = int(L_sched[jj])
            a = xe[off[jj]:off[jj + 1]].reshape(Lj, P, F)      # [l, p, f]
            a = a.transpose(2, 0, 1)                           # [f, l, p] = [k,f128,l,p]
            parts.append(np.ascontiguousarray(a).reshape(-1))
        xeT = np.concatenate(parts)
        del xe

        # own-node x for a_dst, blocked layout [j][f(128)][k(2)][n(128)]
        rr = row2node[cc * NPC:(cc + 1) * NPC]
        mm = rr >= 0
        xo = np.zeros((NPC, F), dtype=bf16)
        xo[mm] = x_bf[rr[mm]]
        xoT = np.ascontiguousarray(
            xo.reshape(NB, P, 2, P).transpose(0, 3, 2, 1)).reshape(NB, P, 2 * P)

        in_maps.append({
            "xeT": xeT,
            "xoT": xoT,
            "Wt_ext": Wt_ext_f8,
            "Wtd": Wtd_bf,
            "bias_rep": bias_rep,
        })
    return in_maps, L_sched, S, row2node


def _build_program(L_sched, S, reps=1, BX=4, BH=5, BA=2, B2=4, NG=1):
    nc = bacc.Bacc("TRN2", target_bir_lowering=False, debug=False,
                   enable_asserts=False, num_devices=NCORES)
    dt = mybir.dt

    xeT = nc.dram_tensor("xeT", [S * 2 * P], dt.float8e4, kind="ExternalInput").ap()
    xoT = nc.dram_tensor("xoT", [NB, P, 2 * P], dt.bfloat16, kind="ExternalInput").ap()
    Wt_ext = nc.dram_tensor("Wt_ext", [F, TD], dt.float8e4, kind="ExternalInput").ap()
    Wtd = nc.dram_tensor("Wtd", [F, H], dt.bfloat16, kind="ExternalInput").ap()
    bias_rep = nc.dram_tensor("bias_rep", [P, HC], dt.float32, kind="ExternalInput").ap()
    out = nc.dram_tensor("out", [NPC, HC], dt.float32, kind="ExternalOutput").ap()

    AF = mybir.ActivationFunctionType
    OP = mybir.AluOpType
    GP7 = 7  # l-groups of 7 share one 504-col psum bank

    with tile.TileContext(nc) as tc:
        with (
            tc.tile_pool(name="const", bufs=1) as constp,
            tc.tile_pool(name="resid", bufs=1) as residp,
        ):
            wt0 = constp.tile([P, TD], dt.float8e4)
            nc.sync.dma_start(wt0[:], Wt_ext[0:P, :])
            wt1 = constp.tile([P, TD], dt.float8e4)
            nc.sync.dma_start(wt1[:], Wt_ext[P:2 * P, :])
            wtd0 = constp.tile([P, H], dt.bfloat16)
            nc.sync.dma_start(wtd0[:], Wtd[0:P, :])
            wtd1 = constp.tile([P, H], dt.bfloat16)
            nc.sync.dma_start(wtd1[:], Wtd[P:2 * P, :])
            bias_t = constp.tile([P, HC], dt.float32)
            nc.sync.dma_start(bias_t[:], bias_rep[:])
            ident = constp.tile([P, P], dt.bfloat16)
            make_identity(nc, ident[:])

            adst_own = residp.tile([P, NB * H], dt.bfloat16)

            for _rep in range(reps):

                # ---------------- phase 1: a_dst for own nodes ----------------
                with (
                    tc.tile_pool(name="p1", bufs=3) as p1,
                    tc.tile_pool(name="p1ps", bufs=2, space="PSUM") as p1ps,
                ):
                    G1 = 8
                    for t0 in range(0, NB, G1):
                        nb4 = min(G1, NB - t0)
                        xot = p1.tile([P, nb4 * 2 * P], dt.bfloat16, tag="xot")
                        nc.sync.dma_start(
                            xot[:].rearrange("p (t c) -> p t c", c=2 * P),
                            xoT[t0:t0 + nb4].rearrange("t p c -> p t c"))
                        aps = p1ps.tile([P, nb4 * H], dt.float32, space="PSUM")
                        for b in range(nb4):
                            bo = b * 2 * P
                            nc.tensor.matmul(aps[:, b * H:(b + 1) * H],
                                             lhsT=xot[:, bo:bo + P], rhs=wtd0[:],
                                             start=True, stop=False,
                                             skip_group_check=True)
                            nc.tensor.matmul(aps[:, b * H:(b + 1) * H],
                                             lhsT=xot[:, bo + P:bo + 2 * P], rhs=wtd1[:],
                                             start=False, stop=True,
                                             skip_group_check=True)
                        with nc.allow_low_precision(reason="bf16 a_dst store; fp32 accum in PSUM"):
                            nc.vector.tensor_copy(
                                out=adst_own[:, t0 * H:(t0 + nb4) * H], in_=aps[:])

                # ---------------- phase 2: edge blocks ----------------
                with (
                    tc.tile_pool(name="p2x", bufs=BX) as p2x,
                    tc.tile_pool(name="p2", bufs=B2) as p2,
                    tc.tile_pool(name="p2f", bufs=4) as p2f,
                    tc.tile_pool(name="heps", bufs=BH, space="PSUM") as hepsp,
                    tc.tile_pool(name="aggps", bufs=BA, space="PSUM") as aggpsp,
                ):
                    xoff = 0
                    fin0 = 0
                    vst = None
                    CH = NG * GP7  # l-chunk: at most NG heps psum banks live at a time
                    for jb in range(NB):
                        L = int(L_sched[jb])
                        LP = L * P
                        xt = p2x.tile([P, 2 * LP], dt.float8e4, tag="xt")
                        # one DMA for both 128-row f-halves of the contiguous
                        # [256, L*P] block: dims (p, k, c)
                        nc.sync.dma_start(
                            xt[:].rearrange("p (k c) -> p k c", k=2),
                            xeT[xoff:xoff + 2 * P * LP]
                                .rearrange("(k p c) -> p k c", k=2, p=P))
                        xta = xt[:, 0:LP]
                        xtb = xt[:, LP:2 * LP]
                        xoff += 2 * P * LP
                        agg = aggpsp.tile([P, TD], dt.float32, space="PSUM", tag="agg")
                        for ch0 in range(0, L, CH):
                            cl = min(CH, L - ch0)
                            ngrp = (cl + GP7 - 1) // GP7
                            heps = []
                            for grp in range(ngrp):
                                gl = min(GP7, cl - grp * GP7)
                                ps = hepsp.tile([P, gl * TD], dt.float32, space="PSUM", tag="heps")
                                heps.append((ps, gl))
                            logits = p2.tile([P, cl, H], dt.float32, tag="logits")
                            for grp, (ps, gl) in enumerate(heps):
                                for li in range(gl):
                                    l = ch0 + grp * GP7 + li
                                    nc.tensor.matmul(ps[:, li * TD:(li + 1) * TD],
                                                     lhsT=xta[:, l * P:(l + 1) * P], rhs=wt0[:],
                                                     start=(li == 0), stop=False, skip_group_check=True)
                                    nc.tensor.matmul(ps[:, li * TD:(li + 1) * TD],
                                                     lhsT=xtb[:, l * P:(l + 1) * P], rhs=wt1[:],
                                                     start=False, stop=(li == gl - 1), skip_group_check=True)
                                # logits[l, h] = asrc + adst (adst bcast over l)
                                nc.vector.tensor_tensor(
                                    out=logits[:, grp * GP7:grp * GP7 + gl, :],
                                    in0=ps[:].rearrange("p (l d) -> p l d", d=TD)[:, :, HC:TD],
                                    in1=adst_own[:, jb * H:(jb + 1) * H].unsqueeze(1).to_broadcast([P, gl, H]),
                                    op=OP.add)
                            # exp(leaky(z)) = max(exp(z), exp(0.2 z))
                            ex1 = p2.tile([P, cl, H], dt.bfloat16, tag="ex1")
                            nc.scalar.activation(ex1[:], logits[:], AF.Exp)
                            ex2 = p2.tile([P, cl, H], dt.bfloat16, tag="ex2")
                            nc.scalar.activation(ex2[:], logits[:], AF.Exp, scale=NEG_SLOPE)
                            w = p2.tile([P, cl, TD], dt.bfloat16, tag="w")
                            nc.vector.tensor_tensor(out=w[:, :, HC:TD], in0=ex1[:], in1=ex2[:], op=OP.max)
                            # w[:, :, 0:64] = h * expe (per-head broadcast)
                            for grp, (ps, gl) in enumerate(heps):
                                nc.vector.tensor_tensor(
                                    out=w[:, grp * GP7:grp * GP7 + gl, 0:HC].rearrange("p l (h c) -> p l h c", c=C),
                                    in0=ps[:].rearrange("p (l d) -> p l d", d=TD)[:, :, 0:HC].rearrange("p l (h c) -> p l h c", c=C),
                                    in1=w[:, grp * GP7:grp * GP7 + gl, HC:TD].unsqueeze(3).to_broadcast([P, gl, H, C]),
                                    op=OP.mult)
                            # aggregate: psum += I.T @ w_l  -> [m | s]
                            for li in range(cl):
                                l = ch0 + li
                                nc.tensor.matmul(agg[:], lhsT=ident[:], rhs=w[:, li, :],
                                                 start=(l == 0), stop=(l == L - 1), skip_group_check=True)
                        # finalize: v = m/s + bias staged; log_softmax batched
                        if jb % FIN == 0:
                            fin0 = jb
                            kf = min(FIN, NB - fin0)
                            vst = p2f.tile([P, kf * HC], dt.float32, tag="vst")
                        jl = jb - fin0
                        srecip = p2f.tile([P, H], dt.float32, tag="srecip")
                        nc.vector.reciprocal(srecip[:], agg[:, HC:TD])
                        onorm = p2f.tile([P, HC], dt.float32, tag="onorm")
                        nc.vector.tensor_tensor(
                            out=onorm[:].rearrange("p (h c) -> p h c", c=C),
                            in0=agg[:, 0:HC].rearrange("p (h c) -> p h c", c=C),
                            in1=srecip[:].unsqueeze(2).to_broadcast([P, H, C]),
                            op=OP.mult)
                        nc.gpsimd.tensor_tensor(
                            out=vst[:, jl * HC:(jl + 1) * HC],
                            in0=onorm[:], in1=bias_t[:], op=OP.add)
                        if jb == NB - 1 or (jb + 1) % FIN == 0:
                            kf = jb - fin0 + 1
                            et = p2f.tile([P, kf * HC], dt.bfloat16, tag="et")
                            nc.scalar.activation(et[:], vst[:, 0:kf * HC], AF.Exp)
                            sm = p2f.tile([P, kf], dt.float32, tag="sm")
                            nc.vector.tensor_reduce(
                                sm[:].unsqueeze(2),
                                et[:].rearrange("p (f d) -> p f d", d=HC),
                                axis=mybir.AxisListType.X, op=OP.add)
                            ln = p2f.tile([P, kf], dt.float32, tag="ln")
                            nc.scalar.activation(ln[:], sm[:], AF.Ln)
                            fin = p2f.tile([P, kf * HC], dt.float32, tag="fin")
                            nc.vector.tensor_tensor(
                                out=fin[:].rearrange("p (f d) -> p f d", d=HC),
                                in0=vst[:, 0:kf * HC].rearrange("p (f d) -> p f d", d=HC),
                                in1=ln[:].unsqueeze(2).to_broadcast([P, kf, HC]),
                                op=OP.subtract)
                            nc.sync.dma_start(
                                out[fin0 * P:(jb + 1) * P, :]
                                    .rearrange("(f p) d -> p f d", p=P),
                                fin[:].rearrange("p (f d) -> p f d", d=HC))

    nc.compile()
    return nc


def kernel(x, edge_index, W, att_src, att_dst, bias):
    in_maps, L_sched, S, row2node = _host_prep(x, edge_index, W, att_src, att_dst, bias)
    nc = _build_program(L_sched, S)
    res = run_bass_kernel_spmd(nc, in_maps, core_ids=list(range(NCORES)))
    out_full = np.empty((N, HC), dtype=np.float32)
    for cc in range(NCORES):
        o = res.results[cc]["out"]
        rr = row2node[cc * NPC:(cc + 1) * NPC]
        m = rr >= 0
        out_full[rr[m]] = o[m]
    return out_full

